# revision 23
# baseline (speedup 1.0000x reference)
"""Trainium2 Bass kernel for the fused GNN message-passing block.

Reference computation (per batch b):
    h = silu(x @ W1 + b1) @ W2 + b2                       # [K, C]
    out[q, d, c] = sum_k mask[q,k] * ev[q,k,d] * ef[q,k,c] * h[k,c]

Sharding: data-parallel over (b, q-half) -> 8 cores, each core handles
one b (of 4) and 64 of the 128 q values.  Tiny MLP weights replicated.

v9 strategy (measured ~34.3us vs the 36.4us v3 baseline; floor model:
6.4us fixed NEFF startup + ~3.9us to first DMA byte + stream + tail):
  - ef ships as INT8 with per-(q,k)-row quantization scales folded into
    access_mask on the host (mask' = mask * rowmax/127).  SWDGE
    (gpsimd) DMAs cast int8->bf16 inline (write-side ~380 GB/s), so
    HBM reads halve to 2.1 MB/core; SBUF sees bf16 and the main DVE
    multiply keeps the fast 16-bit 2x_1P mode ((58+FD/2)/0.96ns —
    the hard cap; gpsimd TT offload poisons concurrent DVE 4x).
  - The weight packs ride the SAME SWDGE ring AHEAD of the ef chunks:
    ring FIFO guarantees they win the SDMA bandwidth race (on a
    separate queue they crawl at ~20% share behind ef and stall the
    h8 chain ~3us; a dep-gate fails because Tile hoists dep-free
    descriptor-gen instructions).  All SWDGE descriptor generation
    sits before the first TT — Q7 ring writes concurrent with DVE
    slow TTs ~25%; the cast stream itself does not.
  - MLP: pack A (W1/xT/b1) lands before pack B (W2/b2); one PSUM bank
    for h1; fused Silu on ACT (no sigmoid+mult pair); h8 seed +
    doubling replication all on DVE (cross-engine hops cost ~0.4us).
  - w = (mask*ev)^T is built as a dense [K,QSH,3] fp32 tile (no 32-col
    padding: LDWEIGHTS cost scales with columns) and cast on ACT; em
    rides the scalar ring (tiny, must not queue behind ef).
  - Main loop: 8 groups of 8q; per q one [128k,3]x[128k,256] matmul
    into 4 concurrent 32-col PE tiles (tile_position=(0,32s)); chunk
    tail sizes 8,4,4 shorten the post-stream trail.
  - Output accumulates in o_all [128, NG, 512] and leaves as FOUR
    contiguous 8KB-per-partition-row DMAs (one per PSUM col-group s,
    3 contiguous partitions each), 2 sync + 2 scalar.  The v3
    baseline's 12 tiny phased writes cost ~11us of tail; slicing the
    group dim fragments descriptors to 1KB and crawls (measured).

The walrus build in this container accepts at most ONE sync wait per
instruction; _split_multiwaits() legalizes the finalized BIR.
"""

import numpy as np
import ml_dtypes

import concourse.bass as bass
import concourse.mybir as mybir
import concourse.tile as tile
from concourse.bass import ds, ts
from concourse.bass_utils import run_bass_kernel_spmd

B, Q, K, D, C = 4, 128, 128, 3, 256
N_CORES = 8
QSH = Q // 2  # 64 q rows per core
QB = 8        # q values per compute group (one PSUM bank of output)
NG = QSH // QB
F32 = mybir.dt.float32
F16 = mybir.dt.bfloat16
I8 = mybir.dt.int8
BF16 = ml_dtypes.bfloat16

EF_INT8 = True

# ef chunking: (q_count, kind).  'i8' chunks ship as int8 and ride the
# gpsimd (SWDGE) ring with an inline int8->bf16 cast (halves HBM
# reads; solo cast DMA measured ~380 GB/s on the SBUF-write side,
# which is the binding resource).  'f16' chunks would ride sync, but
# v5 showed two ef queues just split the same SDMA bandwidth while
# starving the small weight DMAs, so all ef goes int8 on one ring.
# ALL shipped ef is pre-scaled by 127/rowmax on the host (dequant
# scale folded into access_mask).
CHUNKS = [(16, "i8"), (16, "i8"), (16, "i8"), (8, "i8"), (4, "i8"), (4, "i8")]
# groups whose ef*h multiply runs on gpsimd instead of DVE (DVE is the
# compute pacer at ~9.3us of TT; gpsimd idles after descriptor gen)
GPSIMD_TT_GROUPS = ()
WARMUP_MMS = 0

# mlp packs (bf16 cols).  Pack A feeds the h1 matmuls and lands
# first; pack B (W2/b2) is needed ~1us later.
PKA_W1 = 0         # [128, 2, 256]  W1[(o p), n] -> p, o, n
PKA_XT = 512       # [128, 2, 128]  x^T[(o p), k] -> p, o, k
PKA_B1 = 768       # [1, 256] b1 on partition 0
PKA_F = 1024
PKB_W2 = 0         # [128, 2, 256]
PKB_B2 = 512       # [1, 256] b2 on partition 0
PKB_F = 768

# evmask pack free-dim layout (fp32 cols)
EM_EV = 0          # [128, 3, 64]   ev^T[k, d, q]
EM_MASK = 192      # [128, 64]      (mask*scale)^T[k, q]
EM_F = 256

_NC_CACHE = {}


def _split_multiwaits(nc):
    """Legalize for the 1-sync-wait-per-instruction walrus: hoist all but
    the last wait of each instruction onto single-wait NOPs placed just
    before it on the same engine queue."""
    n = 0
    for f in nc.m.functions:
        for bb in f.blocks:
            out = []
            for inst in bb.instructions:
                si = inst.sync_info
                if si is not None and si.on_wait and len(si.on_wait) > 1:
                    waits = list(si.on_wait)
                    for w in waits[:-1]:
                        n += 1
                        nop = mybir.InstNoOp(
                            name=f"{inst.name}-wsplit{n}", ins=[], outs=[]
                        )
                        nop.engine = inst.engine
                        nop.sync_info = mybir.SyncInfo(on_wait=[w], on_update=[])
                        out.append(nop)
                    inst.sync_info = mybir.SyncInfo(
                        on_wait=[waits[-1]], on_update=list(si.on_update)
                    )
                out.append(inst)
            bb.instructions = out
    return nc


def _build_nc(split=True):
    nc = bass.Bass()

    n_i8 = sum(qn for qn, kind in CHUNKS if kind == "i8") if EF_INT8 else 0
    n_f16 = QSH - n_i8
    if n_i8:
        ef8_d = nc.declare_dram_parameter("efT8", [K, n_i8, C], I8, isOutput=False)
    if n_f16:
        ef16_d = nc.declare_dram_parameter("efT16", [K, n_f16, C], F16, isOutput=False)
    mlpa_d = nc.declare_dram_parameter("mlpA", [128, PKA_F], F16, isOutput=False)
    mlpb_d = nc.declare_dram_parameter("mlpB", [128, PKB_F], F16, isOutput=False)
    em_d = nc.declare_dram_parameter("evmask", [K, EM_F], F32, isOutput=False)
    outa_d = nc.declare_dram_parameter(
        "orawA", [4, D, NG - 1, 2 * C], F16, isOutput=True
    )
    outb_d = nc.declare_dram_parameter("orawB", [4, D, 2 * C], F16, isOutput=True)

    with tile.TileContext(nc) as tc:
        with (
            tc.tile_pool(name="const", bufs=1) as cpool,
            tc.tile_pool(name="efp", bufs=1) as efpool,
            tc.tile_pool(name="pprep", bufs=1, space="PSUM") as pprep,
            tc.tile_pool(name="pout", bufs=5, space="PSUM") as pout,
        ):
            ones_sb = cpool.tile([1, 128], F16)
            nc.vector.memset(ones_sb[:], 1.0)

            # ---- input DMAs.  sync ring: mlp pack (head critical
            # path); scalar ring: em (tiny, must not queue behind the
            # ef stream); gpsimd ring (SWDGE): all ef cast chunks. ----
            mlpa_sb = cpool.tile([128, PKA_F], F16)
            nc.gpsimd.dma_start(mlpa_sb[:], mlpa_d[:, :])
            mlpb_sb = cpool.tile([128, PKB_F], F16)
            nc.gpsimd.dma_start(mlpb_sb[:], mlpb_d[:, :])
            em_sb = cpool.tile([K, EM_F], F32)
            nc.scalar.dma_start(em_sb[:], em_d[:, :])

            ef_slots = []
            q0 = 0
            o8 = 0
            o16 = 0
            for ci, (qn, kind) in enumerate(CHUNKS):
                slot = efpool.tile([K, qn, C], F16, tag=f"ef{ci}", name=f"ef{ci}")
                ef_slots.append((slot, q0, qn))
                if EF_INT8 and kind == "i8":
                    nc.gpsimd.dma_start(slot[:], ef8_d[:, ds(o8, qn), :])
                    o8 += qn
                else:
                    nc.sync.dma_start(slot[:], ef16_d[:, ds(o16, qn), :])
                    o16 += qn
                q0 += qn


            w1v = mlpa_sb[:, PKA_W1 : PKA_W1 + 512].rearrange("p (o n) -> p o n", n=C)
            xTv = mlpa_sb[:, PKA_XT : PKA_XT + 256].rearrange("p (o k) -> p o k", k=128)
            b1v = mlpa_sb[0:1, PKA_B1 : PKA_B1 + C]
            w2v = mlpb_sb[:, PKB_W2 : PKB_W2 + 512].rearrange("p (o n) -> p o n", n=C)
            b2v = mlpb_sb[0:1, PKB_B2 : PKB_B2 + C]
            evTv = em_sb[:, EM_EV : EM_EV + 192].rearrange("p (d q) -> p d q", q=QSH)
            maskTv = em_sb[:, EM_MASK : EM_MASK + QSH]

            # ---- MLP h1 matmuls (fp32), single PSUM bank ----
            h1T_ps = pprep.tile([128, 2 * 128], F32, tag="h1", name="h1T")
            for dh in range(2):
                nc.tensor.matmul(
                    h1T_ps[:, ds(128 * dh, 128)], w1v[:, 0, ts(dh, 128)],
                    xTv[:, 0, :], start=True, stop=False,
                )
                nc.tensor.matmul(
                    h1T_ps[:, ds(128 * dh, 128)], w1v[:, 1, ts(dh, 128)],
                    xTv[:, 1, :], start=False, stop=False,
                )
                nc.tensor.matmul(
                    h1T_ps[:, ds(128 * dh, 128)], b1v[:, ts(dh, 128)], ones_sb[:],
                    start=False, stop=True, tile_position=(0, 0),
                )

            # ---- w32 = (mask * ev)^T in fp32, dense [K, QSH, 3]; all
            # cols written so no memset.  Strided fp32 DVE writes are
            # safe (only strided bf16 writes corrupt neighbors).  On
            # the DVE queue this sits before the silu TT (em lands
            # early on the scalar ring). ----
            w32_sb = cpool.tile([128, QSH, D], F32)
            for d in range(D):
                nc.vector.tensor_copy(w32_sb[:, :, d], evTv[:, d, :])
            nc.vector.tensor_tensor(
                w32_sb[:],
                w32_sb[:],
                maskTv[:, :, None].to_broadcast([K, QSH, D]),
                mybir.AluOpType.mult,
            )

            # fused SiLU on ACT first (head critical path), then the
            # w cast (needed later, ~9.5us).
            h1sT_sb = cpool.tile([128, 256], F16)
            nc.scalar.activation(
                h1sT_sb[:], h1T_ps[:], mybir.ActivationFunctionType.Silu
            )
            h_ps = pprep.tile([128, C], F32, tag="hps", name="h_ps")
            nc.tensor.matmul(
                h_ps[:], h1sT_sb[:, 0:128], w2v[:, 0], start=True, stop=False
            )
            nc.tensor.matmul(
                h_ps[:], h1sT_sb[:, 128:256], w2v[:, 1], start=False, stop=False
            )
            nc.tensor.matmul(
                h_ps[:], ones_sb[:], b2v[:], start=False, stop=True,
                tile_position=(0, 0),
            )

            # cast h -> bf16 and replicate x8 along q: ACT and DVE seed
            # the first two copies in parallel, DVE doubles the rest
            h8_sb = cpool.tile([128, QB * C], F16)
            nc.scalar.copy(out=h8_sb[:, :C], in_=h_ps[:])
            nc.vector.tensor_copy(h8_sb[:, C : 2 * C], h_ps[:])
            nc.vector.tensor_copy(h8_sb[:, 2 * C : 4 * C], h8_sb[:, : 2 * C])
            nc.vector.tensor_copy(h8_sb[:, 4 * C : 8 * C], h8_sb[:, : 4 * C])
            h8v = h8_sb[:, : QB * C].rearrange("p (j c) -> p j c", c=C)
            w_sb = cpool.tile([128, QSH, D], F16)
            nc.scalar.copy(out=w_sb[:], in_=w32_sb[:])

            # ---- main loop: groups of 8 q; all groups drain into one
            # SBUF accumulator; output leaves as 4 DMAs at the end ----
            def chunk_of(q):
                acc = 0
                for slot, cq0, qn in ef_slots:
                    if cq0 <= q < cq0 + qn:
                        return slot, q - cq0
                    acc += qn
                raise AssertionError

            oa_sb = cpool.tile([128, NG - 1, 2 * C], F16)
            ob_sb = cpool.tile([128, 2 * C], F16)
            for g in range(NG):
                last = g == NG - 1
                halves = 2 if last else 1
                ps = pout.tile([128, 2 * C], F32, tag="opsum", name="ps")
                for hv in range(halves):
                    js = range(hv * QB // halves, (hv + 1) * QB // halves)
                    nq = len(js)
                    slot, off = chunk_of(g * QB + js.start)
                    nc.vector.tensor_tensor(
                        slot[:, ds(off, nq), :],
                        slot[:, ds(off, nq), :],
                        h8v[:, :nq, :],
                        mybir.AluOpType.mult,
                    )
                    for j in js:
                        f, s = j // 4, j % 4
                        q = g * QB + j
                        sl, qoff = chunk_of(q)
                        nc.tensor.matmul(
                            ps[ds(32 * s, D), ds(C * f, C)],
                            w_sb[:, q, :],
                            sl[:, qoff, :],
                            start=True,
                            stop=True,
                            tile_position=(0, 32 * s),
                        )
                    cw = (2 * C) // halves
                    dst = (
                        ob_sb[:, ds(hv * cw, cw)]
                        if last
                        else oa_sb[:, g, ds(hv * cw, cw)]
                    )
                    nc.scalar.copy(out=dst, in_=ps[:, ds(hv * cw, cw)])
                if g == NG - 2:
                    # wave A: groups 0..NG-2 leave on the idle sync ring
                    # while the last group computes
                    for s in range(4):
                        nc.sync.dma_start(
                            outa_d[s, :, :, :], oa_sb[ds(32 * s, D), :, :]
                        )

            # wave B: only the last group's 3KB-per-s rows sit on the tail
            for s in range(4):
                eng = (nc.sync, nc.scalar, nc.gpsimd, nc.scalar)[s]
                eng.dma_start(outb_d[s, :, :], ob_sb[ds(32 * s, D), :])

    return _split_multiwaits(nc) if split else nc


def _get_nc():
    if "nc" not in _NC_CACHE:
        _NC_CACHE["nc"] = _build_nc()
    return _NC_CACHE["nc"]


def _in_maps(inputs):
    x = np.asarray(inputs["x"], dtype=np.float32)
    ev = np.asarray(inputs["ev"], dtype=np.float32)
    ef = np.asarray(inputs["ef"], dtype=np.float32)
    am = np.asarray(inputs["access_mask"], dtype=np.float32)
    W1 = np.asarray(inputs["W1"], dtype=np.float32)
    b1 = np.asarray(inputs["b1"], dtype=np.float32)
    W2 = np.asarray(inputs["W2"], dtype=np.float32)
    b2 = np.asarray(inputs["b2"], dtype=np.float32)

    if EF_INT8:
        # normalize ef rows to +-127; dequant scale folds into mask.
        # i8 chunks round to int8, f16 chunks keep bf16 precision.
        rowmax = np.abs(ef).max(axis=-1, keepdims=True)
        rowmax = np.maximum(rowmax, 1e-30)
        ef = ef * (127.0 / rowmax)
        ef_q = np.clip(np.rint(ef), -127, 127).astype(np.int8)
        am = am * (rowmax[..., 0] / 127.0)
        i8_qs = []
        f16_qs = []
        q0 = 0
        for qn, kind in CHUNKS:
            (i8_qs if kind == "i8" else f16_qs).append((q0, qn))
            q0 += qn

    # shared weight packs A (W1/xT/b1) and B (W2/b2)
    packa = np.zeros((128, PKA_F), dtype=np.float32)
    packa[:, PKA_W1 : PKA_W1 + 512] = W1.reshape(2, 128, C).transpose(1, 0, 2).reshape(
        128, 512
    )
    packa[0, PKA_B1 : PKA_B1 + C] = b1
    packb = np.zeros((128, PKB_F), dtype=np.float32)
    packb[:, PKB_W2 : PKB_W2 + 512] = W2.reshape(2, 128, C).transpose(1, 0, 2).reshape(
        128, 512
    )
    packb[0, PKB_B2 : PKB_B2 + C] = b2
    packb16 = packb.astype(BF16)

    maps = []
    for core in range(N_CORES):
        b, qh = core // 2, core % 2
        sl = slice(qh * QSH, (qh + 1) * QSH)
        pk = packa.copy()
        # xT: x[b]^T [C, K] -> [128, 2, 128] (p = c % 128, o = c // 128)
        pk[:, PKA_XT : PKA_XT + 256] = (
            x[b].T.reshape(2, 128, 128).transpose(1, 0, 2).reshape(128, 256)
        )
        em = np.empty((K, EM_F), dtype=np.float32)
        em[:, EM_EV : EM_EV + 192] = (
            ev[b, sl].transpose(1, 2, 0).reshape(K, 192)
        )
        em[:, EM_MASK : EM_MASK + QSH] = am[b, sl].T
        m = {"mlpA": pk.astype(BF16), "mlpB": packb16, "evmask": em}
        if EF_INT8:
            eft8 = ef_q[b, sl].transpose(1, 0, 2)   # [K, QSH, C] int8
            eft16 = ef[b, sl].transpose(1, 0, 2)    # [K, QSH, C] f32 (normalized)
            m["efT8"] = np.ascontiguousarray(
                np.concatenate([eft8[:, q0 : q0 + qn] for q0, qn in i8_qs], axis=1)
            )
            if f16_qs:
                m["efT16"] = np.ascontiguousarray(
                    np.concatenate(
                        [eft16[:, q0 : q0 + qn] for q0, qn in f16_qs], axis=1
                    ).astype(BF16)
                )
        else:
            m["efT16"] = np.ascontiguousarray(
                ef[b, sl].transpose(1, 0, 2).astype(BF16)
            )
        maps.append(m)
    return maps


def _gather(results):
    out = np.empty((B, Q, D, C), dtype=np.float32)
    for core in range(N_CORES):
        b, qh = core // 2, core % 2
        oa = results[core]["orawA"].astype(np.float32)  # [4, 3, NG-1, 512]
        ob = results[core]["orawB"].astype(np.float32)  # [4, 3, 512]
        oraw = np.concatenate([oa, ob[:, :, None, :]], axis=2)
        arr = oraw.reshape(4, D, NG, 2, C)  # [s, d, g, f, c]
        out[b, qh * QSH : (qh + 1) * QSH] = (
            arr.transpose(2, 3, 0, 1, 4).reshape(QSH, D, C)
        )
    return out


def _run(inputs, trace=False, **kwargs):
    nc = _get_nc()
    res = run_bass_kernel_spmd(
        nc, _in_maps(inputs), list(range(N_CORES)), trace=trace, **kwargs
    )
    return _gather(res.results), res


def kernel(**inputs) -> np.ndarray:
    out, _ = _run(inputs, trace=False)
    return out


# revision 25
# speedup vs baseline: 1.1696x; 1.1696x over previous
"""Trainium2 Bass kernel for the fused GNN message-passing block.

Reference computation (per batch b):
    h = silu(x @ W1 + b1) @ W2 + b2                       # [K, C]
    out[q, d, c] = sum_k mask[q,k] * ev[q,k,d] * ef[q,k,c] * h[k,c]

Sharding: data-parallel over (b, q-half) -> 8 cores, each core handles
one b (of 4) and 64 of the 128 q values.  Tiny MLP weights replicated.

v9 strategy (measured ~34.3us vs the 36.4us v3 baseline; floor model:
6.4us fixed NEFF startup + ~3.9us to first DMA byte + stream + tail):
  - ef ships as INT8 with per-(q,k)-row quantization scales folded into
    access_mask on the host (mask' = mask * rowmax/127).  SWDGE
    (gpsimd) DMAs cast int8->bf16 inline (write-side ~380 GB/s), so
    HBM reads halve to 2.1 MB/core; SBUF sees bf16 and the main DVE
    multiply keeps the fast 16-bit 2x_1P mode ((58+FD/2)/0.96ns —
    the hard cap; gpsimd TT offload poisons concurrent DVE 4x).
  - The weight packs ride the SAME SWDGE ring AHEAD of the ef chunks:
    ring FIFO guarantees they win the SDMA bandwidth race (on a
    separate queue they crawl at ~20% share behind ef and stall the
    h8 chain ~3us; a dep-gate fails because Tile hoists dep-free
    descriptor-gen instructions).  All SWDGE descriptor generation
    sits before the first TT — Q7 ring writes concurrent with DVE
    slow TTs ~25%; the cast stream itself does not.
  - MLP: pack A (W1/xT/b1) lands before pack B (W2/b2); one PSUM bank
    for h1; fused Silu on ACT (no sigmoid+mult pair); h8 seed +
    doubling replication all on DVE (cross-engine hops cost ~0.4us).
  - w = (mask*ev)^T is built as a dense [K,QSH,3] fp32 tile (no 32-col
    padding: LDWEIGHTS cost scales with columns) and cast on ACT; em
    rides the scalar ring (tiny, must not queue behind ef).
  - Main loop: 8 groups of 8q; per q one [128k,3]x[128k,256] matmul
    into 4 concurrent 32-col PE tiles (tile_position=(0,32s)); chunk
    tail sizes 8,4,4 shorten the post-stream trail.
  - Output accumulates in o_all [128, NG, 512] and leaves as FOUR
    contiguous 8KB-per-partition-row DMAs (one per PSUM col-group s,
    3 contiguous partitions each), 2 sync + 2 scalar.  The v3
    baseline's 12 tiny phased writes cost ~11us of tail; slicing the
    group dim fragments descriptors to 1KB and crawls (measured).

The walrus build in this container accepts at most ONE sync wait per
instruction; _split_multiwaits() legalizes the finalized BIR.
"""

import numpy as np
import ml_dtypes

import concourse.bass as bass
import concourse.mybir as mybir
import concourse.tile as tile
from concourse.bass import ds, ts
from concourse.bass_utils import run_bass_kernel_spmd

B, Q, K, D, C = 4, 128, 128, 3, 256
N_CORES = 8
QSH = Q // 2  # 64 q rows per core
QB = 8        # q values per compute group (one PSUM bank of output)
NG = QSH // QB
F32 = mybir.dt.float32
F16 = mybir.dt.bfloat16
I8 = mybir.dt.int8
BF16 = ml_dtypes.bfloat16

EF_INT8 = True

# ef chunking: (q_count, kind).  'i8' chunks ship as int8 and ride the
# gpsimd (SWDGE) ring with an inline int8->bf16 cast (halves HBM
# reads; solo cast DMA measured ~380 GB/s on the SBUF-write side,
# which is the binding resource).  'f16' chunks would ride sync, but
# v5 showed two ef queues just split the same SDMA bandwidth while
# starving the small weight DMAs, so all ef goes int8 on one ring.
# ALL shipped ef is pre-scaled by 127/rowmax on the host (dequant
# scale folded into access_mask).
CHUNKS = [(16, "i8"), (16, "i8"), (16, "i8"), (8, "i8"), (4, "i8"), (4, "i8")]
# groups whose ef*h multiply runs on gpsimd instead of DVE (DVE is the
# compute pacer at ~9.3us of TT; gpsimd idles after descriptor gen)
GPSIMD_TT_GROUPS = ()
WARMUP_MMS = 0

# mlp packs (bf16 cols).  Pack A feeds the h1 matmuls and lands
# first; pack B (W2/b2) is needed ~1us later.
PKA_W1 = 0         # [128, 2, 256]  W1[(o p), n] -> p, o, n
PKA_XT = 512       # [128, 2, 128]  x^T[(o p), k] -> p, o, k
PKA_B1 = 768       # [1, 256] b1 on partition 0
PKA_F = 1024
PKB_W2 = 0         # [128, 2, 256]
PKB_B2 = 512       # [1, 256] b2 on partition 0
PKB_F = 768

# evmask pack free-dim layout (fp32 cols)
EM_EV = 0          # [128, 3, 64]   ev^T[k, d, q]
EM_MASK = 192      # [128, 64]      (mask*scale)^T[k, q]
EM_F = 256

_NC_CACHE = {}


def _split_multiwaits(nc):
    """Legalize for the 1-sync-wait-per-instruction walrus: hoist all but
    the last wait of each instruction onto single-wait NOPs placed just
    before it on the same engine queue."""
    n = 0
    for f in nc.m.functions:
        for bb in f.blocks:
            out = []
            for inst in bb.instructions:
                si = inst.sync_info
                if si is not None and si.on_wait and len(si.on_wait) > 1:
                    waits = list(si.on_wait)
                    for w in waits[:-1]:
                        n += 1
                        nop = mybir.InstNoOp(
                            name=f"{inst.name}-wsplit{n}", ins=[], outs=[]
                        )
                        nop.engine = inst.engine
                        nop.sync_info = mybir.SyncInfo(on_wait=[w], on_update=[])
                        out.append(nop)
                    inst.sync_info = mybir.SyncInfo(
                        on_wait=[waits[-1]], on_update=list(si.on_update)
                    )
                out.append(inst)
            bb.instructions = out
    return nc


def _build_nc(split=True):
    nc = bass.Bass()

    n_i8 = sum(qn for qn, kind in CHUNKS if kind == "i8") if EF_INT8 else 0
    n_f16 = QSH - n_i8
    if n_i8:
        ef8_d = nc.declare_dram_parameter("efT8", [K, n_i8, C], I8, isOutput=False)
    if n_f16:
        ef16_d = nc.declare_dram_parameter("efT16", [K, n_f16, C], F16, isOutput=False)
    mlpa_d = nc.declare_dram_parameter("mlpA", [128, PKA_F], F16, isOutput=False)
    mlpb_d = nc.declare_dram_parameter("mlpB", [128, PKB_F], F16, isOutput=False)
    em_d = nc.declare_dram_parameter("evmask", [K, EM_F], F32, isOutput=False)
    out_d = nc.declare_dram_parameter("oraw", [4, D, NG, 2 * C], F16, isOutput=True)

    with tile.TileContext(nc) as tc:
        with (
            tc.tile_pool(name="const", bufs=1) as cpool,
            tc.tile_pool(name="efp", bufs=1) as efpool,
            tc.tile_pool(name="pprep", bufs=1, space="PSUM") as pprep,
            tc.tile_pool(name="pout", bufs=5, space="PSUM") as pout,
        ):
            ones_sb = cpool.tile([1, 128], F16)
            nc.vector.memset(ones_sb[:], 1.0)

            # ---- input DMAs.  sync ring: mlp pack (head critical
            # path); scalar ring: em (tiny, must not queue behind the
            # ef stream); gpsimd ring (SWDGE): all ef cast chunks. ----
            mlpa_sb = cpool.tile([128, PKA_F], F16)
            nc.gpsimd.dma_start(mlpa_sb[:], mlpa_d[:, :])
            mlpb_sb = cpool.tile([128, PKB_F], F16)
            nc.gpsimd.dma_start(mlpb_sb[:], mlpb_d[:, :])
            em_sb = cpool.tile([K, EM_F], F32)
            nc.scalar.dma_start(em_sb[:], em_d[:, :])

            ef_slots = []
            q0 = 0
            o8 = 0
            o16 = 0
            for ci, (qn, kind) in enumerate(CHUNKS):
                slot = efpool.tile([K, qn, C], F16, tag=f"ef{ci}", name=f"ef{ci}")
                ef_slots.append((slot, q0, qn))
                if EF_INT8 and kind == "i8":
                    nc.gpsimd.dma_start(slot[:], ef8_d[:, ds(o8, qn), :])
                    o8 += qn
                else:
                    nc.sync.dma_start(slot[:], ef16_d[:, ds(o16, qn), :])
                    o16 += qn
                q0 += qn


            w1v = mlpa_sb[:, PKA_W1 : PKA_W1 + 512].rearrange("p (o n) -> p o n", n=C)
            xTv = mlpa_sb[:, PKA_XT : PKA_XT + 256].rearrange("p (o k) -> p o k", k=128)
            b1v = mlpa_sb[0:1, PKA_B1 : PKA_B1 + C]
            w2v = mlpb_sb[:, PKB_W2 : PKB_W2 + 512].rearrange("p (o n) -> p o n", n=C)
            b2v = mlpb_sb[0:1, PKB_B2 : PKB_B2 + C]
            evTv = em_sb[:, EM_EV : EM_EV + 192].rearrange("p (d q) -> p d q", q=QSH)
            maskTv = em_sb[:, EM_MASK : EM_MASK + QSH]

            # ---- MLP h1 matmuls (fp32), single PSUM bank ----
            h1T_ps = pprep.tile([128, 2 * 128], F32, tag="h1", name="h1T")
            for dh in range(2):
                nc.tensor.matmul(
                    h1T_ps[:, ds(128 * dh, 128)], w1v[:, 0, ts(dh, 128)],
                    xTv[:, 0, :], start=True, stop=False,
                )
                nc.tensor.matmul(
                    h1T_ps[:, ds(128 * dh, 128)], w1v[:, 1, ts(dh, 128)],
                    xTv[:, 1, :], start=False, stop=False,
                )
                nc.tensor.matmul(
                    h1T_ps[:, ds(128 * dh, 128)], b1v[:, ts(dh, 128)], ones_sb[:],
                    start=False, stop=True, tile_position=(0, 0),
                )

            # ---- w32 = (mask * ev)^T in fp32, dense [K, QSH, 3]; all
            # cols written so no memset.  Strided fp32 DVE writes are
            # safe (only strided bf16 writes corrupt neighbors).  On
            # the DVE queue this sits before the silu TT (em lands
            # early on the scalar ring). ----
            w32_sb = cpool.tile([128, QSH, D], F32)
            for d in range(D):
                nc.vector.tensor_copy(w32_sb[:, :, d], evTv[:, d, :])
            nc.vector.tensor_tensor(
                w32_sb[:],
                w32_sb[:],
                maskTv[:, :, None].to_broadcast([K, QSH, D]),
                mybir.AluOpType.mult,
            )

            # fused SiLU on ACT first (head critical path), then the
            # w cast (needed later, ~9.5us).
            h1sT_sb = cpool.tile([128, 256], F16)
            nc.scalar.activation(
                h1sT_sb[:], h1T_ps[:], mybir.ActivationFunctionType.Silu
            )
            w_sb = cpool.tile([128, QSH, D], F16)
            nc.scalar.copy(out=w_sb[:], in_=w32_sb[:])
            h_ps = pprep.tile([128, C], F32, tag="hps", name="h_ps")
            nc.tensor.matmul(
                h_ps[:], h1sT_sb[:, 0:128], w2v[:, 0], start=True, stop=False
            )
            nc.tensor.matmul(
                h_ps[:], h1sT_sb[:, 128:256], w2v[:, 1], start=False, stop=False
            )
            nc.tensor.matmul(
                h_ps[:], ones_sb[:], b2v[:], start=False, stop=True,
                tile_position=(0, 0),
            )

            # cast h -> bf16 and replicate x8 along q (doubling copies),
            # all on DVE so the chain has no cross-engine hops
            h8_sb = cpool.tile([128, QB * C], F16)
            nc.vector.tensor_copy(h8_sb[:, :C], h_ps[:])
            nc.vector.tensor_copy(h8_sb[:, C : 2 * C], h8_sb[:, :C])
            nc.vector.tensor_copy(h8_sb[:, 2 * C : 4 * C], h8_sb[:, : 2 * C])
            nc.vector.tensor_copy(h8_sb[:, 4 * C : 8 * C], h8_sb[:, : 4 * C])
            h8v = h8_sb[:, : QB * C].rearrange("p (j c) -> p j c", c=C)

            # ---- main loop: groups of 8 q; all groups drain into one
            # SBUF accumulator; output leaves as 4 DMAs at the end ----
            def chunk_of(q):
                acc = 0
                for slot, cq0, qn in ef_slots:
                    if cq0 <= q < cq0 + qn:
                        return slot, q - cq0
                    acc += qn
                raise AssertionError

            o_all = cpool.tile([128, NG, 2 * C], F16)
            for g in range(NG):
                halves = 2 if g == NG - 1 else 1
                ps = pout.tile([128, 2 * C], F32, tag="opsum", name="ps")
                for hv in range(halves):
                    js = range(hv * QB // halves, (hv + 1) * QB // halves)
                    nq = len(js)
                    slot, off = chunk_of(g * QB + js.start)
                    nc.vector.tensor_tensor(
                        slot[:, ds(off, nq), :],
                        slot[:, ds(off, nq), :],
                        h8v[:, :nq, :],
                        mybir.AluOpType.mult,
                    )
                    for j in js:
                        f, s = j // 4, j % 4
                        q = g * QB + j
                        sl, qoff = chunk_of(q)
                        nc.tensor.matmul(
                            ps[ds(32 * s, D), ds(C * f, C)],
                            w_sb[:, q, :],
                            sl[:, qoff, :],
                            start=True,
                            stop=True,
                            tile_position=(0, 32 * s),
                        )
                    nc.scalar.copy(
                        out=o_all[:, g, ds(hv * (2 * C) // halves, (2 * C) // halves)],
                        in_=ps[:, ds(hv * (2 * C) // halves, (2 * C) // halves)],
                    )

            # output: one DMA per PSUM col-group s — [3 partitions,
            # NG*512] contiguous 8KB-per-partition slices
            for s in range(4):
                eng = (nc.sync, nc.scalar, nc.sync, nc.scalar)[s]
                eng.dma_start(out_d[s, :, :, :], o_all[ds(32 * s, D), :, :])

    return _split_multiwaits(nc) if split else nc


def _get_nc():
    if "nc" not in _NC_CACHE:
        _NC_CACHE["nc"] = _build_nc()
    return _NC_CACHE["nc"]


def _in_maps(inputs):
    x = np.asarray(inputs["x"], dtype=np.float32)
    ev = np.asarray(inputs["ev"], dtype=np.float32)
    ef = np.asarray(inputs["ef"], dtype=np.float32)
    am = np.asarray(inputs["access_mask"], dtype=np.float32)
    W1 = np.asarray(inputs["W1"], dtype=np.float32)
    b1 = np.asarray(inputs["b1"], dtype=np.float32)
    W2 = np.asarray(inputs["W2"], dtype=np.float32)
    b2 = np.asarray(inputs["b2"], dtype=np.float32)

    if EF_INT8:
        # normalize ef rows to +-127; dequant scale folds into mask.
        # i8 chunks round to int8, f16 chunks keep bf16 precision.
        rowmax = np.abs(ef).max(axis=-1, keepdims=True)
        rowmax = np.maximum(rowmax, 1e-30)
        ef = ef * (127.0 / rowmax)
        ef_q = np.clip(np.rint(ef), -127, 127).astype(np.int8)
        am = am * (rowmax[..., 0] / 127.0)
        i8_qs = []
        f16_qs = []
        q0 = 0
        for qn, kind in CHUNKS:
            (i8_qs if kind == "i8" else f16_qs).append((q0, qn))
            q0 += qn

    # shared weight packs A (W1/xT/b1) and B (W2/b2)
    packa = np.zeros((128, PKA_F), dtype=np.float32)
    packa[:, PKA_W1 : PKA_W1 + 512] = W1.reshape(2, 128, C).transpose(1, 0, 2).reshape(
        128, 512
    )
    packa[0, PKA_B1 : PKA_B1 + C] = b1
    packb = np.zeros((128, PKB_F), dtype=np.float32)
    packb[:, PKB_W2 : PKB_W2 + 512] = W2.reshape(2, 128, C).transpose(1, 0, 2).reshape(
        128, 512
    )
    packb[0, PKB_B2 : PKB_B2 + C] = b2
    packb16 = packb.astype(BF16)

    maps = []
    for core in range(N_CORES):
        b, qh = core // 2, core % 2
        sl = slice(qh * QSH, (qh + 1) * QSH)
        pk = packa.copy()
        # xT: x[b]^T [C, K] -> [128, 2, 128] (p = c % 128, o = c // 128)
        pk[:, PKA_XT : PKA_XT + 256] = (
            x[b].T.reshape(2, 128, 128).transpose(1, 0, 2).reshape(128, 256)
        )
        em = np.empty((K, EM_F), dtype=np.float32)
        em[:, EM_EV : EM_EV + 192] = (
            ev[b, sl].transpose(1, 2, 0).reshape(K, 192)
        )
        em[:, EM_MASK : EM_MASK + QSH] = am[b, sl].T
        m = {"mlpA": pk.astype(BF16), "mlpB": packb16, "evmask": em}
        if EF_INT8:
            eft8 = ef_q[b, sl].transpose(1, 0, 2)   # [K, QSH, C] int8
            eft16 = ef[b, sl].transpose(1, 0, 2)    # [K, QSH, C] f32 (normalized)
            m["efT8"] = np.ascontiguousarray(
                np.concatenate([eft8[:, q0 : q0 + qn] for q0, qn in i8_qs], axis=1)
            )
            if f16_qs:
                m["efT16"] = np.ascontiguousarray(
                    np.concatenate(
                        [eft16[:, q0 : q0 + qn] for q0, qn in f16_qs], axis=1
                    ).astype(BF16)
                )
        else:
            m["efT16"] = np.ascontiguousarray(
                ef[b, sl].transpose(1, 0, 2).astype(BF16)
            )
        maps.append(m)
    return maps


def _gather(results):
    out = np.empty((B, Q, D, C), dtype=np.float32)
    for core in range(N_CORES):
        b, qh = core // 2, core % 2
        oraw = results[core]["oraw"].astype(np.float32)  # [4, 3, NG, 512]
        arr = oraw.reshape(4, D, NG, 2, C)  # [s, d, g, f, c]
        out[b, qh * QSH : (qh + 1) * QSH] = (
            arr.transpose(2, 3, 0, 1, 4).reshape(QSH, D, C)
        )
    return out


def _run(inputs, trace=False, **kwargs):
    nc = _get_nc()
    res = run_bass_kernel_spmd(
        nc, _in_maps(inputs), list(range(N_CORES)), trace=trace, **kwargs
    )
    return _gather(res.results), res


def kernel(**inputs) -> np.ndarray:
    out, _ = _run(inputs, trace=False)
    return out
